# revision 56
# baseline (speedup 1.0000x reference)
"""Trainium2 Bass kernel for LpAlignEntropyLoss (B=2048, D=128, 2 views).

loss = mean_i ||z0_i - z1_i + eps||  -  0.5 * sum_v mean_i [ logsumexp_{j!=i}(-||zv_i - zv_j + eps||) - log(B-1) ]

Strategy (8 NeuronCores, batch-row sharded, 256 rows/core, symmetric-half):
  dist^2[i,j] = n_i + n_j - 2 * z_i . z_j   (matmul trick, bf16 TensorE)
  The 16x16 grid of 128x128 blocks is covered once using symmetry: each
  row-chunk p computes column blocks at ring distance 0..7 (a [128,1024]
  PSUM slab) plus its distance-8 block (shared [128,4,128] "pex" slab,
  Sqrt'd AFTER the main chain so the last main Sqrt lands early).
  Row sums cover distances 0..7; distances 1..8 are recovered on the host
  from per-block column sums (stationary-operand matmuls, PE cost ~ free
  size = 1) of the transposed blocks: colsum_j(block{q -> q+b}) equals
  the missing rowsum piece of chunk q+b by symmetry.  Every row sees all
  2047 partners while each elementwise engine only touches 9/16 of the
  distance matrix.

  Engine split:
   - PE: all matmuls; n_j via lhsT=negh (all -0.5) x rhs=sq (=zt*zt),
     n_i rows (pex) via lhsT=sq-chunk x rhs=negh; diag mask via
     ident x (-BIG ident); column sums; warm-up matmuls for pstate.
   - ACT: ONLY Sqrt (+Copy staging) -> a single activation-table load,
     hoisted into the idle head by an early dummy Sqrt.
   - DVE: sq, exp via Schraudolph's bit trick truncated to 16 bits
     (exp(x) ~ bitcast_bf16(i16(A*x + B)), one tensor_scalar each, 2x
     SBUF mode), row sums via tensor_scalar+accum (4x bf16 mode),
     staging copies.
  Only 4 input DMAs + 1 output (HWDGE descriptor-gen is ~625 ns and
  globally serialized).  Host finishes the O(B) tail: assemble rowsums,
  log, sqrt, means.

eps=1e-8 is below fp32 ulp of every operand magnitude here; dropping it
is exact at fp32 resolution.  BIG=1800 puts the masked diagonal at
d=60: exp(-60) underflows to 0 in bf16 and stays in-range for the
int16 exp trick.
"""
import numpy as np
import ml_dtypes
from contextlib import ExitStack

B = 2048
D = 128
N_CORES = 8
R = B // N_CORES          # 256 rows per core
W = 1280                  # local columns held per core (10 chunks)
MAIN = 1024               # main slab width (ring distance 0..7)
BIG = 1800.0              # diag mask: d_ii = 60
TAU = 1.0
LOG_NM1 = float(np.log(B - 1))
N_WARMUP = 21             # PE pstate warm-up matmuls
EXP_A = 184.66496         # Schraudolph/16: exp(x) ~ bitcast_bf16(i16(A*x+B))
EXP_B = 16249.146         # (127*2^23 - C)/2^16, bias-calibrated for our d's

_cache: dict = {}


def _build():
    import concourse.tile as tile
    from concourse import bacc, mybir
    import concourse.mybir as mb

    f32 = mybir.dt.float32
    i16 = mybir.dt.int16
    bf16 = mybir.dt.bfloat16
    AF = mybir.ActivationFunctionType

    nc = bacc.Bacc("TRN2", target_bir_lowering=False, debug=False,
                   num_devices=N_CORES)

    zt_d = [nc.dram_tensor(f"zt{v}", [D, W], bf16, kind="ExternalInput").ap()
            for v in (0, 1)]
    consts_d = nc.dram_tensor("consts", [128, 256], bf16,
                              kind="ExternalInput").ap()
    out_d = nc.dram_tensor("out", [128, 39], f32, kind="ExternalOutput").ap()

    with tile.TileContext(nc) as tc, ExitStack() as ctx:
        consts = ctx.enter_context(tc.tile_pool(name="consts", bufs=1))
        ztp = ctx.enter_context(tc.tile_pool(name="ztp", bufs=1))
        psum = ctx.enter_context(tc.tile_pool(name="psum", bufs=1, space="PSUM"))
        distp = ctx.enter_context(tc.tile_pool(name="distp", bufs=1))
        ep = ctx.enter_context(tc.tile_pool(name="ep", bufs=1))
        outp = ctx.enter_context(tc.tile_pool(name="outp", bufs=1))

        # ---- input DMAs on the SP HWDGE queue ----
        sb_zt = []
        for v in (0, 1):
            t_ = ztp.tile([D, W], bf16, tag=f"zt{v}", name=f"sb_zt{v}")
            sb_zt.append(t_)
        sb_c = consts.tile([128, 256], bf16, tag="consts", name="sb_c")
        nc.sync.dma_start(sb_zt[0][:, 0:1024], zt_d[0][:, 0:1024])
        nc.sync.dma_start(sb_zt[0][:, 1024:1280], zt_d[0][:, 1024:1280])
        nc.sync.dma_start(sb_c[:], consts_d)
        nc.sync.dma_start(sb_zt[1][:], zt_d[1])
        ident = sb_c[:, 0:128]
        ibig = sb_c[:, 128:256]

        ones = consts.tile([128, 128], bf16, tag="ones", name="ones")
        nc.vector.memset(ones[:], 1.0)
        negh = consts.tile([128, 128], bf16, tag="negh", name="negh")
        nc.vector.memset(negh[:], -0.5)

        # PSUM: 3 rotating 2-bank [128,1024] slabs (warm-up junk + 4 main
        # slabs + late b=8 colsum columns all share this tag) + 1-bank pex
        # + 1-bank outP = 8 banks.
        outP = psum.tile([128, 34], f32, tag="out", name="outP")
        pex = psum.tile([128, 4, 128], f32, tag="pex", name="pex")

        # PE pstate warm-up into the first rotating slab buffer
        warmP = psum.tile([128, MAIN], f32, tag="slab", bufs=3, name="warmP")
        for _ in range(N_WARMUP):
            nc.tensor.matmul(warmP[:, 0:128], ones[:], ones[:],
                             start=True, stop=True)

        # dummy early Sqrt: the sqrt-table load attaches to its (trivial)
        # waits and runs in the idle head instead of gating the first Sqrt.
        dummy = outp.tile([128, 1], f32, tag="dummy", name="dummy")
        nc.scalar.activation(dummy[:], ones[:, 0:1], AF.Sqrt,
                             bias=0.0, scale=1.0)

        # ---- sq = zt*zt (DVE); n_i colnorms -> outP cols 30..33 ----
        sq = []
        for v in (0, 1):
            s_ = outp.tile([128, W], bf16, tag=f"sq{v}", name=f"sq{v}")
            sq.append(s_)
        nc.vector.tensor_mul(sq[0][:, 0:512], sb_zt[0][:, 0:512],
                             sb_zt[0][:, 0:512])
        nc.vector.tensor_mul(sq[0][:, 512:1024], sb_zt[0][:, 512:1024],
                             sb_zt[0][:, 512:1024])
        nc.vector.tensor_mul(sq[0][:, 1024:1280], sb_zt[0][:, 1024:1280],
                             sb_zt[0][:, 1024:1280])
        nc.vector.tensor_mul(sq[1][:, 0:512], sb_zt[1][:, 0:512],
                             sb_zt[1][:, 0:512])
        nc.vector.tensor_mul(sq[1][:, 512:1024], sb_zt[1][:, 512:1024],
                             sb_zt[1][:, 512:1024])
        nc.vector.tensor_mul(sq[1][:, 1024:1280], sb_zt[1][:, 1024:1280],
                             sb_zt[1][:, 1024:1280])

        nrow = outp.tile([128, 4], f32, tag="nrow", name="nrow")
        for v in (0, 1):
            for t in range(2):
                nc.tensor.matmul(outP[:, 30 + 2 * v + t:31 + 2 * v + t],
                                 sq[v][:, t * 128:(t + 1) * 128], ones[:, 0:1],
                                 start=True, stop=True)
            nc.vector.tensor_copy(nrow[:, 2 * v:2 * v + 2],
                                  outP[:, 30 + 2 * v:32 + 2 * v])

        # ---- main slabs: ring distance 0..7 per (view, chunk) ----
        dists = {}
        for v in (0, 1):
            for t in range(2):
                P = psum.tile([128, MAIN], f32, tag="slab", bufs=3, name="P")
                w0 = t * 128
                lhsT = sb_zt[v][:, t * 128:(t + 1) * 128]
                for s in range(2):
                    sl = slice(s * 512, (s + 1) * 512)
                    wsl = slice(w0 + s * 512, w0 + (s + 1) * 512)
                    nc.tensor.matmul(P[:, sl], lhsT, sb_zt[v][:, wsl],
                                     start=True, stop=False)
                    nc.tensor.matmul(P[:, sl], negh[:], sq[v][:, wsl],
                                     start=False, stop=(s != 0))
                nc.tensor.matmul(P[:, 0:128], ident, ibig,
                                 start=False, stop=True)
                idx = v * 2 + t
                dist = distp.tile([128, MAIN], f32, tag=f"dist{idx}",
                                  name=f"dist{idx}")
                nc.scalar.activation(dist[:], P[:], AF.Sqrt,
                                     bias=nrow[:, idx:idx + 1], scale=-2.0)
                dists[idx] = dist

        # ---- pex: the four distance-8 blocks, Sqrt AFTER the main chain ----
        for q in range(4):
            v, t = q // 2, q % 2
            csl = slice(1024 + t * 128, 1024 + (t + 1) * 128)
            lhsT = sb_zt[v][:, t * 128:(t + 1) * 128]
            nc.tensor.matmul(pex[:, q, :], lhsT, sb_zt[v][:, csl],
                             start=True, stop=False)
            nc.tensor.matmul(pex[:, q, :], negh[:], sq[v][:, csl],
                             start=False, stop=False)
            nc.tensor.matmul(pex[:, q, :], sq[v][:, t * 128:(t + 1) * 128],
                             negh[:, 0:128], start=False, stop=True)
        dist_ex = distp.tile([128, 4, 128], f32, tag="dist_ex", name="dist_ex")
        nc.scalar.activation(dist_ex[:], pex[:], AF.Sqrt, bias=0.0, scale=-2.0)

        # ---- align term: ||z0_i - z1_i||^2 -> outP cols 0/1 ----
        adiff = outp.tile([128, 256], bf16, tag="adiff", name="adiff")
        nc.vector.tensor_sub(adiff[:], sb_zt[0][:, 0:256], sb_zt[1][:, 0:256])
        asq = outp.tile([128, 256], bf16, tag="asq", name="asq")
        nc.vector.tensor_mul(asq[:], adiff[:], adiff[:])
        for h in range(2):
            nc.tensor.matmul(outP[:, h:h + 1],
                             asq[:, h * 128:(h + 1) * 128], ones[:, 0:1],
                             start=True, stop=True)

        export = outp.tile([128, 39], f32, tag="export", name="export")
        rdump = ep.tile([128, MAIN], bf16, tag="rdump", name="rdump")

        # ---- Schraudolph Exp on DVE + row sums + column sums ----
        efs = {}
        for idx in range(4):
            Ei = ep.tile([128, MAIN], i16, tag=f"ei{idx}", name=f"ei{idx}")
            nc.vector.tensor_scalar(Ei[:], dists[idx][:], -EXP_A, EXP_B,
                                    mb.AluOpType.mult, mb.AluOpType.add)
            efs[idx] = Ei.bitcast(bf16)
        eix = ep.tile([128, 4, 128], i16, tag="eix", name="eix")
        nc.vector.tensor_scalar(eix[:], dist_ex[:], -EXP_A, EXP_B,
                                mb.AluOpType.mult, mb.AluOpType.add)
        efx = eix.bitcast(bf16)

        # row sums (export cols: r0=34 r1=35 r2a=36 r3=37 r2b=38; r2 is
        # split so no DVE op overlaps the last dists becoming ready)
        for idx, spans in ((0, [(0, MAIN, 34)]), (1, [(0, MAIN, 35)]),
                           (2, [(0, 512, 36), (512, MAIN, 38)]),
                           (3, [(0, MAIN, 37)])):
            for a, b_, col in spans:
                nc.vector.tensor_scalar(rdump[:, a:b_], efs[idx][:, a:b_],
                                        1.0, 0.0, mb.AluOpType.mult,
                                        mb.AluOpType.add,
                                        accum_out=export[:, col:col + 1])
        for idx in range(4):
            for b in range(1, 8):
                col = 2 + idx * 7 + (b - 1)
                nc.tensor.matmul(outP[:, col:col + 1],
                                 efs[idx][:, b * 128:(b + 1) * 128],
                                 ones[:, 0:1], start=True, stop=True)
        # b=8 colsums from the pex quarters into a late rotating-slab tile
        late = psum.tile([128, MAIN], f32, tag="slab", bufs=3, name="late")
        for q in range(4):
            nc.tensor.matmul(late[:, q:q + 1], efx[:, q, :], ones[:, 0:1],
                             start=True, stop=True)

        # staging copies on the idle ACT engine (Copy needs no table load)
        nc.scalar.copy(export[:, 0:30], outP[:, 0:30])
        nc.scalar.copy(export[:, 30:34], late[:, 0:4])
        nc.sync.dma_start(out_d, export[:])

    nc.compile()
    return nc


def _prep_inputs(z0: np.ndarray, z1: np.ndarray):
    """Per-core input maps: rotate columns so core c's rows come first."""
    bf = ml_dtypes.bfloat16
    zs = [np.ascontiguousarray(z0, np.float32), np.ascontiguousarray(z1, np.float32)]
    eye = np.eye(128, dtype=np.float32)
    consts = np.concatenate([eye, -BIG * eye], axis=1).astype(bf)  # [128, 256]
    in_maps = []
    for c in range(N_CORES):
        order = (np.arange(W) + c * R) % B
        m = {"consts": consts}
        for v in (0, 1):
            zr = zs[v][order]                                    # [W, D] rotated
            m[f"zt{v}"] = np.ascontiguousarray(zr.T).astype(bf)  # [D, W]
        in_maps.append(m)
    return in_maps


def kernel(z0: np.ndarray, z1: np.ndarray) -> np.ndarray:
    from concourse.bass_utils import run_bass_kernel_spmd

    if "nc" not in _cache:
        _cache["nc"] = _build()
    nc = _cache["nc"]

    in_maps = _prep_inputs(z0, z1)
    res = run_bass_kernel_spmd(nc, in_maps, core_ids=list(range(N_CORES)))

    rowsums = np.zeros((2, B), np.float64)   # [view, global row]
    alignsq = np.empty((B,), np.float64)
    for c in range(N_CORES):
        out = res.results[c]["out"].astype(np.float64)   # [128, 39]
        rcol = {0: out[:, 34], 1: out[:, 35],
                2: out[:, 36] + out[:, 38], 3: out[:, 37]}
        for v in (0, 1):
            for t in range(2):
                idx = v * 2 + t
                own = ((2 * c + t) % 16) * 128
                # own row sums cover ring distances 0..7
                rowsums[v, own:own + 128] += rcol[idx]
                # received column sums (distances 1..8, transposed rows)
                for b in range(1, 9):
                    g = ((2 * c + t + b) % 16) * 128
                    col = 2 + idx * 7 + (b - 1) if b < 8 else 30 + idx
                    rowsums[v, g:g + 128] += out[:, col]
        alignsq[c * R:c * R + 128] = out[:, 0]
        alignsq[c * R + 128:c * R + 256] = out[:, 1]

    align_loss = np.sqrt(alignsq).mean()
    lme = np.log(rowsums) - LOG_NM1             # [2, B]
    entropy_loss = lme.mean()
    return np.float32(align_loss - entropy_loss)


# revision 64
# speedup vs baseline: 1.0243x; 1.0243x over previous
"""Trainium2 Bass kernel for LpAlignEntropyLoss (B=2048, D=128, 2 views).

loss = mean_i ||z0_i - z1_i + eps||  -  0.5 * sum_v mean_i [ logsumexp_{j!=i}(-||zv_i - zv_j + eps||) - log(B-1) ]

Strategy (8 NeuronCores, batch-row sharded, 256 rows/core, symmetric-half):
  dist^2[i,j] = n_i + n_j - 2 * z_i . z_j   (matmul trick, bf16 TensorE)
  The 16x16 grid of 128x128 blocks is covered once using symmetry: each
  row-chunk p computes column blocks at ring distance 0..7 (a [128,1024]
  PSUM slab) plus its distance-8 block (shared [128,4,128] "pex" slab,
  Sqrt'd AFTER the main chain so the last main Sqrt lands early).
  Row sums cover distances 0..7; distances 1..8 are recovered on the host
  from per-block column sums (stationary-operand matmuls, PE cost ~ free
  size = 1) of the transposed blocks: colsum_j(block{q -> q+b}) equals
  the missing rowsum piece of chunk q+b by symmetry.  Every row sees all
  2047 partners while each elementwise engine only touches 9/16 of the
  distance matrix.

  Engine split:
   - PE: all matmuls; n_j via lhsT=negh (all -0.5) x rhs=sq (=zt*zt),
     n_i rows (pex) via lhsT=sq-chunk x rhs=negh; diag mask via
     ident x (-BIG ident); column sums; warm-up matmuls for pstate.
   - ACT: ONLY Sqrt (+Copy staging) -> a single activation-table load,
     hoisted into the idle head by an early dummy Sqrt.
   - DVE: sq, exp via Schraudolph's bit trick truncated to 16 bits
     (exp(x) ~ bitcast_bf16(i16(A*x + B)), one tensor_scalar each, 2x
     SBUF mode), row sums via tensor_scalar+accum (4x bf16 mode),
     staging copies.
  Only 4 input DMAs + 1 output (HWDGE descriptor-gen is ~625 ns and
  globally serialized).  Host finishes the O(B) tail: assemble rowsums,
  log, sqrt, means.

eps=1e-8 is below fp32 ulp of every operand magnitude here; dropping it
is exact at fp32 resolution.  BIG=1800 puts the masked diagonal at
d=60: exp(-60) underflows to 0 in bf16 and stays in-range for the
int16 exp trick.
"""
import numpy as np
import ml_dtypes
from contextlib import ExitStack

B = 2048
D = 128
N_CORES = 8
R = B // N_CORES          # 256 rows per core
W = 1280                  # local columns held per core (10 chunks)
MAIN = 1024               # main slab width (ring distance 0..7)
BIG = 1800.0              # diag mask: d_ii = 60
TAU = 1.0
LOG_NM1 = float(np.log(B - 1))
N_WARMUP = 21             # PE pstate warm-up matmuls
EXP_A = 184.66496         # Schraudolph/16: exp(x) ~ bitcast_bf16(i16(A*x+B))
EXP_B = 16249.146         # (127*2^23 - C)/2^16, bias-calibrated for our d's

_cache: dict = {}


def _build():
    import concourse.tile as tile
    from concourse import bacc, mybir
    import concourse.mybir as mb

    f32 = mybir.dt.float32
    i16 = mybir.dt.int16
    bf16 = mybir.dt.bfloat16
    AF = mybir.ActivationFunctionType

    nc = bacc.Bacc("TRN2", target_bir_lowering=False, debug=False,
                   num_devices=N_CORES)

    zt_d = [nc.dram_tensor(f"zt{v}", [D, W], bf16, kind="ExternalInput").ap()
            for v in (0, 1)]
    consts_d = nc.dram_tensor("consts", [128, 256], bf16,
                              kind="ExternalInput").ap()
    out_d = nc.dram_tensor("out", [128, 35], f32, kind="ExternalOutput").ap()
    outx_d = nc.dram_tensor("outx", [128, 4, 128], f32,
                            kind="ExternalOutput").ap()

    with tile.TileContext(nc) as tc, ExitStack() as ctx:
        consts = ctx.enter_context(tc.tile_pool(name="consts", bufs=1))
        ztp = ctx.enter_context(tc.tile_pool(name="ztp", bufs=1))
        psum = ctx.enter_context(tc.tile_pool(name="psum", bufs=1, space="PSUM"))
        distp = ctx.enter_context(tc.tile_pool(name="distp", bufs=1))
        ep = ctx.enter_context(tc.tile_pool(name="ep", bufs=1))
        outp = ctx.enter_context(tc.tile_pool(name="outp", bufs=1))

        # ---- input DMAs on the SP HWDGE queue ----
        sb_zt = []
        for v in (0, 1):
            t_ = ztp.tile([D, W], bf16, tag=f"zt{v}", name=f"sb_zt{v}")
            sb_zt.append(t_)
        sb_c = consts.tile([128, 256], bf16, tag="consts", name="sb_c")
        nc.sync.dma_start(sb_zt[0][:, 0:1024], zt_d[0][:, 0:1024])
        nc.sync.dma_start(sb_zt[0][:, 1024:1280], zt_d[0][:, 1024:1280])
        nc.sync.dma_start(sb_c[:], consts_d)
        nc.sync.dma_start(sb_zt[1][:], zt_d[1])
        ident = sb_c[:, 0:128]
        ibig = sb_c[:, 128:256]

        ones = consts.tile([128, 128], bf16, tag="ones", name="ones")
        nc.vector.memset(ones[:], 1.0)
        negh = consts.tile([128, 128], bf16, tag="negh", name="negh")
        nc.vector.memset(negh[:], -0.5)

        # PSUM: 3 rotating 2-bank [128,1024] slabs (warm-up junk + 4 main
        # slabs + late b=8 colsum columns all share this tag) + 1-bank pex
        # + 1-bank outP = 8 banks.
        outP = psum.tile([128, 34], f32, tag="out", name="outP")
        pex = psum.tile([128, 4, 128], f32, tag="pex", name="pex")

        # PE pstate warm-up into the first rotating slab buffer
        warmP = psum.tile([128, MAIN], f32, tag="slab", bufs=3, name="warmP")
        for _ in range(N_WARMUP):
            nc.tensor.matmul(warmP[:, 0:128], ones[:], ones[:],
                             start=True, stop=True)

        # dummy early Sqrt: the sqrt-table load attaches to its (trivial)
        # waits and runs in the idle head instead of gating the first Sqrt.
        dummy = outp.tile([128, 1], f32, tag="dummy", name="dummy")
        nc.scalar.activation(dummy[:], ones[:, 0:1], AF.Sqrt,
                             bias=0.0, scale=1.0)

        # ---- sq = zt*zt (DVE); n_i colnorms -> outP cols 30..33 ----
        sq = []
        for v in (0, 1):
            s_ = outp.tile([128, W], bf16, tag=f"sq{v}", name=f"sq{v}")
            sq.append(s_)
        nc.vector.tensor_mul(sq[0][:, 0:512], sb_zt[0][:, 0:512],
                             sb_zt[0][:, 0:512])
        nc.vector.tensor_mul(sq[0][:, 512:1024], sb_zt[0][:, 512:1024],
                             sb_zt[0][:, 512:1024])
        nc.vector.tensor_mul(sq[0][:, 1024:1280], sb_zt[0][:, 1024:1280],
                             sb_zt[0][:, 1024:1280])
        nc.vector.tensor_mul(sq[1][:, 0:512], sb_zt[1][:, 0:512],
                             sb_zt[1][:, 0:512])
        nc.vector.tensor_mul(sq[1][:, 512:1024], sb_zt[1][:, 512:1024],
                             sb_zt[1][:, 512:1024])
        nc.vector.tensor_mul(sq[1][:, 1024:1280], sb_zt[1][:, 1024:1280],
                             sb_zt[1][:, 1024:1280])

        nrow = outp.tile([128, 4], f32, tag="nrow", name="nrow")
        for v in (0, 1):
            for t in range(2):
                nc.tensor.matmul(outP[:, 30 + 2 * v + t:31 + 2 * v + t],
                                 sq[v][:, t * 128:(t + 1) * 128], ones[:, 0:1],
                                 start=True, stop=True)
            nc.vector.tensor_copy(nrow[:, 2 * v:2 * v + 2],
                                  outP[:, 30 + 2 * v:32 + 2 * v])

        # ---- main slabs: ring distance 0..7 per (view, chunk) ----
        dists = {}
        for v in (0, 1):
            for t in range(2):
                P = psum.tile([128, MAIN], f32, tag="slab", bufs=3, name="P")
                w0 = t * 128
                lhsT = sb_zt[v][:, t * 128:(t + 1) * 128]
                for s in range(2):
                    sl = slice(s * 512, (s + 1) * 512)
                    wsl = slice(w0 + s * 512, w0 + (s + 1) * 512)
                    nc.tensor.matmul(P[:, sl], lhsT, sb_zt[v][:, wsl],
                                     start=True, stop=False)
                    nc.tensor.matmul(P[:, sl], negh[:], sq[v][:, wsl],
                                     start=False, stop=(s != 0))
                nc.tensor.matmul(P[:, 0:128], ident, ibig,
                                 start=False, stop=True)
                idx = v * 2 + t
                dist = distp.tile([128, MAIN], f32, tag=f"dist{idx}",
                                  name=f"dist{idx}")
                nc.scalar.activation(dist[:], P[:], AF.Sqrt,
                                     bias=nrow[:, idx:idx + 1], scale=-2.0)
                dists[idx] = dist

        # ---- pex: the four distance-8 blocks.  Their ds^2 goes RAW to the
        # host (staged via idle ACT, DMA hidden under the tail); the host
        # does exact sqrt/exp/colsum for these 4x128 values.
        for q in range(4):
            v, t = q // 2, q % 2
            csl = slice(1024 + t * 128, 1024 + (t + 1) * 128)
            lhsT = sb_zt[v][:, t * 128:(t + 1) * 128]
            nc.tensor.matmul(pex[:, q, :], lhsT, sb_zt[v][:, csl],
                             start=True, stop=False)
            nc.tensor.matmul(pex[:, q, :], negh[:], sq[v][:, csl],
                             start=False, stop=False)
            nc.tensor.matmul(pex[:, q, :], sq[v][:, t * 128:(t + 1) * 128],
                             negh[:, 0:128], start=False, stop=True)
        sbx = outp.tile([128, 4, 128], f32, tag="sbx", name="sbx")

        # ---- align term: ||z0_i - z1_i||^2 -> outP cols 0/1 ----
        adiff = outp.tile([128, 256], bf16, tag="adiff", name="adiff")
        nc.vector.tensor_sub(adiff[:], sb_zt[0][:, 0:256], sb_zt[1][:, 0:256])
        asq = outp.tile([128, 256], bf16, tag="asq", name="asq")
        nc.vector.tensor_mul(asq[:], adiff[:], adiff[:])
        for h in range(2):
            nc.tensor.matmul(outP[:, h:h + 1],
                             asq[:, h * 128:(h + 1) * 128], ones[:, 0:1],
                             start=True, stop=True)

        export = outp.tile([128, 35], f32, tag="export", name="export")
        rdump = ep.tile([128, MAIN], bf16, tag="rdump", name="rdump")

        # ---- Schraudolph Exp on DVE + row sums + column sums ----
        efs = {}
        for idx in range(4):
            Ei = ep.tile([128, MAIN], i16, tag=f"ei{idx}", name=f"ei{idx}")
            nc.vector.tensor_scalar(Ei[:], dists[idx][:], -EXP_A, EXP_B,
                                    mb.AluOpType.mult, mb.AluOpType.add)
            efs[idx] = Ei.bitcast(bf16)

        # row sums (export cols: r0=30 r1=31 r2a=32 r3=33 r2b=34; r2 is
        # split so no DVE op overlaps the last dists becoming ready)
        for idx, spans in ((0, [(0, MAIN, 30)]), (1, [(0, MAIN, 31)]),
                           (2, [(0, 512, 32), (512, MAIN, 34)]),
                           (3, [(0, MAIN, 33)])):
            for a, b_, col in spans:
                nc.vector.tensor_scalar(rdump[:, a:b_], efs[idx][:, a:b_],
                                        1.0, 0.0, mb.AluOpType.mult,
                                        mb.AluOpType.add,
                                        accum_out=export[:, col:col + 1])
        for idx in range(4):
            for b in range(1, 8):
                col = 2 + idx * 7 + (b - 1)
                nc.tensor.matmul(outP[:, col:col + 1],
                                 efs[idx][:, b * 128:(b + 1) * 128],
                                 ones[:, 0:1], start=True, stop=True)

        # pex staging + DMA (both hidden under the main tail), then the
        # main staging copy, all on the otherwise idle ACT engine
        nc.scalar.copy(sbx[:], pex[:])
        nc.sync.dma_start(outx_d, sbx[:])
        nc.scalar.copy(export[:, 0:30], outP[:, 0:30])
        nc.sync.dma_start(out_d, export[:])

    nc.compile()
    return nc


def _prep_inputs(z0: np.ndarray, z1: np.ndarray):
    """Per-core input maps: rotate columns so core c's rows come first."""
    bf = ml_dtypes.bfloat16
    zs = [np.ascontiguousarray(z0, np.float32), np.ascontiguousarray(z1, np.float32)]
    eye = np.eye(128, dtype=np.float32)
    consts = np.concatenate([eye, -BIG * eye], axis=1).astype(bf)  # [128, 256]
    in_maps = []
    for c in range(N_CORES):
        order = (np.arange(W) + c * R) % B
        m = {"consts": consts}
        for v in (0, 1):
            zr = zs[v][order]                                    # [W, D] rotated
            m[f"zt{v}"] = np.ascontiguousarray(zr.T).astype(bf)  # [D, W]
        in_maps.append(m)
    return in_maps


def kernel(z0: np.ndarray, z1: np.ndarray) -> np.ndarray:
    from concourse.bass_utils import run_bass_kernel_spmd

    if "nc" not in _cache:
        _cache["nc"] = _build()
    nc = _cache["nc"]

    in_maps = _prep_inputs(z0, z1)
    res = run_bass_kernel_spmd(nc, in_maps, core_ids=list(range(N_CORES)))

    rowsums = np.zeros((2, B), np.float64)   # [view, global row]
    alignsq = np.empty((B,), np.float64)
    for c in range(N_CORES):
        out = res.results[c]["out"].astype(np.float64)   # [128, 35]
        # d8 blocks: raw ds^2/-2 from the device; exact exp+colsum here
        outx = res.results[c]["outx"].astype(np.float64)  # [128, 4, 128]
        ds2x = np.maximum(-2.0 * outx, 0.0)
        ex_colsum = np.exp(-np.sqrt(ds2x)).sum(axis=0)    # [4, 128]
        rcol = {0: out[:, 30], 1: out[:, 31],
                2: out[:, 32] + out[:, 34], 3: out[:, 33]}
        for v in (0, 1):
            for t in range(2):
                idx = v * 2 + t
                own = ((2 * c + t) % 16) * 128
                # own row sums cover ring distances 0..7
                rowsums[v, own:own + 128] += rcol[idx]
                # received column sums (distances 1..7, transposed rows)
                for b in range(1, 8):
                    g = ((2 * c + t + b) % 16) * 128
                    rowsums[v, g:g + 128] += out[:, 2 + idx * 7 + (b - 1)]
                # distance 8 via the host-computed colsum of this chunk's
                # d8 block (symmetry: it feeds chunk 2c+t+8's rows)
                g8 = ((2 * c + t + 8) % 16) * 128
                rowsums[v, g8:g8 + 128] += ex_colsum[idx]
        alignsq[c * R:c * R + 128] = out[:, 0]
        alignsq[c * R + 128:c * R + 256] = out[:, 1]

    align_loss = np.sqrt(alignsq).mean()
    lme = np.log(rowsums) - LOG_NM1             # [2, B]
    entropy_loss = lme.mean()
    return np.float32(align_loss - entropy_loss)
